# revision 1
# baseline (speedup 1.0000x reference)
"""AWQ W4 grouped-dequant matmul on 8 Trainium2 cores.

y = (x / s) @ (w_q * scales).reshape(OUT, IN).T + bias

Column-parallel sharding: each core owns OUT/8 = 1376 output channels
(padded to 1408 = 11*128), x is replicated. Per core the kernel computes
y_shard^T [1408, 2048] = W'[1408, 4096] @ x_bf16[4096, 2048] where the
smoothing division is folded into the weights: W' = (w_q * scales) / s.

Schedule (v4): the PE starts ~15us in (fixed runtime preamble + first
tiles) and chases the W DMA stream; steady state is back-to-back
matmuls.
  - DMA priority order: small tables first, then per k-tile [w8,
    scales, x(chunk 0) piece], then x(1), then y-stores and x(2)/x(3)
    phased behind s_pe progress. All on the SP HWDGE queue in program
    order.
  - Wave 1: 8 PSUM banks accumulate groups (tt=0, ot=0..7) kc-major, so
    each arriving W k-tile feeds 8 matmuls; the wave is paced by the
    ~320GB/s DMA stream (~22MB).
  - W ships as fp8e4m3 (w_q in [-7,7] is exact in e4m3), halving the W
    stream; scales ship replicated x128 in bf16 (a layout move, as in
    the original baseline). DVE dequants each k-tile from a rotating
    fp8 staging slot.
  - fp8 DoubleRow output-split: the last 2 o-tiles per core (rows
    1152..1407, which include the 32 zero-pad rows) are computed
    entirely in fp8: their W' dequantizes to fp8e4m3 (second stt per
    k-tile), x converts to fp8 once per chunk, and the PE runs them as
    self-contained DoubleRow groups ([64, 512] out, 2 k-tiles per
    instruction at 2x rate). The ISA restricts DoubleRow outputs to
    PSUM partitions 0..63 (s3d3_mm_valid_dst_partition), which is why
    the split is by output channels, not by K. Measured rel_err 0.0152
    (vs 0.0033 all-bf16, gate 2e-2) for ~27us less PE time; w_q stays
    exact, only x8 and the dequantized-W' fp8 rounding contribute, on
    16.3% of channels.
  - PSUM eviction runs on the otherwise-idle Scalar engine as
    activation(Identity, bias) into a 4-slot rotating bf16 buffer;
    plain DMAs stream it out. DoubleRow groups evict [64, 512].
  - gpsimd ISA ops (partition_broadcast etc.) do not compile in this
    toolchain ("ISA wrong length" in walrus codegen), hence the
    host-replicated scales.

The toolchain permits AT MOST ONE semaphore wait per instruction. All
waits are standalone engine instructions; every DMA or compute op
carries only its completion increment. DMA completions may reorder
across the 16 engines, so waits only target semaphores whose
increments are causally sequenced (per-transfer sems, terminal values
of bulk chunks, per-slot y sems self-sequenced by the evict/store
cycle, or single-producer engine counters).

Host side does only layout/dtype moves: transpose, pad, shard,
bf16/fp8 casts (w_q ints are exact in fp8e4m3).
"""

import os
from contextlib import ExitStack

import numpy as np

# ---- problem constants (hardcoded per contract) ----
OUT, N_GROUPS, GROUP = 11008, 32, 128
IN = N_GROUPS * GROUP  # 4096
TOKENS = 2048
N_CORES = 8
P = 128
O_SHARD = OUT // N_CORES  # 1376
O_PAD = 1408  # 11 * 128
OT = O_PAD // P  # 11 o-tiles
OBF = 9  # bf16 o-tiles (0..8)
ODR = OT - OBF  # 2 o-tiles in fp8 DoubleRow
DRH = 2 * ODR  # 4 DoubleRow half-groups (64 rows each) per token chunk
O_BF = OBF * P  # 1152 bf16 output columns
KT = IN // P  # 32 k-tiles (== quant groups, GROUP == P)
KTP = KT // 2  # 16 DoubleRow k-pairs
TCH = 512  # tokens per chunk == PSUM bank free size (f32)
NT = TOKENS // TCH  # 4 chunks
NB = 8  # psum banks
WAVE = NB  # wave-1 groups (tt=0, bf16 ot=0..7)
XB = 2  # x chunk buffers
SCB = 4  # w8/scales staging slots
NYS = 4  # y eviction slots

# post-wave schedule: per chunk, bf16 o-tiles then DoubleRow halves.
# (tt=0 contributes only tile 8 + the DR halves; 0..7 ran in wave 1.)
SCHED = []
for _tt in range(NT):
    for _ot in range(OBF):
        if _tt == 0 and _ot < WAVE:
            continue
        SCHED.append(("bf", _tt, _ot))
    for _j2 in range(DRH):
        SCHED.append(("dr", _tt, _j2))
NENT = WAVE + len(SCHED)  # 8 + 44 = 52 evict entries

# matmul count after each entry's last matmul (wave entries are
# kc-major interleaved: group b's last mm is 8*(KT-1)+b+1)
MM_END = [WAVE * (KT - 1) + b + 1 for b in range(WAVE)]
_cum = WAVE * KT
MM_T0 = MM_T1 = None
for _kind, _tt, _i in SCHED:
    _cum += KT if _kind == "bf" else KTP
    MM_END.append(_cum)
    if _tt == 0:
        MM_T0 = _cum  # all tt=0 mms done -> x buffer 0 free
    if _tt == 1:
        MM_T1 = _cum

LAST = {}  # exec_time_ns etc. for the local test harness

_NC_CACHE = {}


def _build_nc():
    import concourse.bass as bass
    from concourse import mybir

    f32 = mybir.dt.float32
    bf16 = mybir.dt.bfloat16
    fp8 = mybir.dt.float8e4

    nc = bass.Bass()
    xT = nc.declare_dram_parameter("xT", [IN, TOKENS], bf16, isOutput=False)
    w8T = nc.declare_dram_parameter("w8T", [IN, O_PAD], fp8, isOutput=False)
    sc_repl = nc.declare_dram_parameter(
        "sc_repl", [N_GROUPS, P, O_PAD], bf16, isOutput=False
    )
    s_cols = nc.declare_dram_parameter("s_cols", [P, KT], f32, isOutput=False)
    bias_cols = nc.declare_dram_parameter("bias_cols", [P, OT], f32, isOutput=False)
    bias_dr = nc.declare_dram_parameter("bias_dr", [64, DRH], f32, isOutput=False)
    yT = nc.declare_dram_parameter("yT", [O_PAD, TOKENS], bf16, isOutput=True)

    with ExitStack() as ctx:
        w_all = ctx.enter_context(nc.sbuf_tensor("w_all", [P, KT * O_BF], bf16))
        w8d2 = ctx.enter_context(nc.sbuf_tensor("w8d2", [P, KTP, 2, 2 * P], fp8))
        xn_all = ctx.enter_context(nc.sbuf_tensor("xn_all", [P, XB * KT * TCH], bf16))
        x8 = ctx.enter_context(nc.sbuf_tensor("x8", [P, XB * KT, TCH], fp8))
        y_sl = ctx.enter_context(nc.sbuf_tensor("y_sl", [P, NYS * TCH], bf16))
        scb = ctx.enter_context(nc.sbuf_tensor("scb", [P, SCB * O_PAD], bf16))
        w8s = ctx.enter_context(nc.sbuf_tensor("w8s", [P, SCB * O_PAD], fp8))
        s_sb = ctx.enter_context(nc.sbuf_tensor("s_sb", [P, KT], f32))
        inv_s = ctx.enter_context(nc.sbuf_tensor("inv_s", [P, KT], f32))
        bias_sb = ctx.enter_context(nc.sbuf_tensor("bias_sb", [P, OT], f32))
        bias_dsb = ctx.enter_context(nc.sbuf_tensor("bias_dsb", [64, DRH], f32))
        ps = [
            ctx.enter_context(nc.psum_tensor(f"ps{i}", [P, TCH], f32))
            for i in range(NB)
        ]
        s_tbl = ctx.enter_context(nc.semaphore("s_tbl"))
        s_wd = [ctx.enter_context(nc.semaphore(f"s_wd{k}")) for k in range(KT)]
        s_x0 = [ctx.enter_context(nc.semaphore(f"s_x0_{k}")) for k in range(KT)]
        s_xc = [ctx.enter_context(nc.semaphore(f"s_xc{t}")) for t in range(1, NT)]
        s_dq = ctx.enter_context(nc.semaphore("s_dq"))
        s_pe = ctx.enter_context(nc.semaphore("s_pe"))
        s_act = ctx.enter_context(nc.semaphore("s_act"))
        s_ys = [ctx.enter_context(nc.semaphore(f"s_ys{j}")) for j in range(NYS)]
        block = ctx.enter_context(nc.Block())

        def wd(kc):
            return w_all[:, kc * O_BF : (kc + 1) * O_BF]

        def xreg(tt, kc):
            o = ((tt % XB) * KT + kc) * TCH
            return xn_all[:, o : o + TCH]

        def x8reg(tt, kc):
            return x8[:, (tt % XB) * KT + kc, :]

        def x8pair(tt, kp):
            b = (tt % XB) * KT
            return x8[:, b + 2 * kp : b + 2 * kp + 2, :]

        def yslot(e):
            o = (e % NYS) * TCH
            return y_sl[:, o : o + TCH]

        # s_dq counting: 1 (recip) + 2 per k-tile (bf16 + DR stt), then
        # 1 per x8 conversion (32 per chunk, chunk order 0,1,2,3)
        def dq_stt(kc):
            return 2 * kc + 3

        DQ_STT = dq_stt(KT - 1)  # 65

        def dq_conv(tt):
            return DQ_STT + 32 * (tt + 1)

        @block.sync
        def _(sync):
            # tables (terminal wait s_tbl >= 48; completions may reorder)
            sync.dma_start(out=s_sb[:, :], in_=s_cols[:, :]).then_inc(s_tbl, 16)
            sync.dma_start(out=bias_sb[:, :], in_=bias_cols[:, :]).then_inc(s_tbl, 16)
            sync.dma_start(out=bias_dsb[:, :], in_=bias_dr[:, :]).then_inc(s_tbl, 16)
            # wave-1 stream: per kc [w8 tile, scales tile, x(0) piece].
            # Staging-slot WAR waits (stt_dr of kc-SCB retired) are
            # pre-satisfied in practice; s_dq is single-producer (DVE).
            for kc in range(KT):
                if kc >= SCB:
                    sync.wait_ge(s_dq, dq_stt(kc - SCB))
                sl = (kc % SCB) * O_PAD
                sync.dma_start(
                    out=w8s[:, sl : sl + O_PAD], in_=w8T[kc * P : (kc + 1) * P, :]
                ).then_inc(s_wd[kc], 16)
                sync.dma_start(
                    out=scb[:, sl : sl + O_PAD], in_=sc_repl[kc, :, :]
                ).then_inc(s_wd[kc], 16)
                sync.dma_start(
                    out=xreg(0, kc), in_=xT[kc * P : (kc + 1) * P, 0:TCH]
                ).then_inc(s_x0[kc], 16)
            # x(1): lands well before tt=1 groups start
            for kc in range(KT):
                sync.dma_start(
                    out=xreg(1, kc), in_=xT[kc * P : (kc + 1) * P, TCH : 2 * TCH]
                ).then_inc(s_xc[0], 16)

            def ystore(e):
                sync.wait_ge(s_act, e + 1)
                if e < WAVE:
                    kind, tt, i = "bf", 0, e
                else:
                    kind, tt, i = SCHED[e - WAVE]
                if kind == "bf":
                    sync.dma_start(
                        out=yT[i * P : (i + 1) * P, tt * TCH : (tt + 1) * TCH],
                        in_=yslot(e),
                    ).then_inc(s_ys[e % NYS], 16)
                else:
                    r0 = O_BF + i * 64
                    sync.dma_start(
                        out=yT[r0 : r0 + 64, tt * TCH : (tt + 1) * TCH],
                        in_=yslot(e)[0:64, :],
                    ).then_inc(s_ys[e % NYS], 16)

            NE_T0 = WAVE + 1 + DRH  # entries for tt=0: wave + tile8 + DR halves
            NE_T = OBF + DRH  # entries per later chunk
            for e in range(NE_T0):
                ystore(e)
            # x(2) reuses buffer 0: all tt=0 matmuls must have retired
            sync.wait_ge(s_pe, MM_T0)
            for kc in range(KT):
                sync.dma_start(
                    out=xreg(2, kc), in_=xT[kc * P : (kc + 1) * P, 2 * TCH : 3 * TCH]
                ).then_inc(s_xc[1], 16)
            for e in range(NE_T0, NE_T0 + NE_T):
                ystore(e)
            # x(3) reuses buffer 1
            sync.wait_ge(s_pe, MM_T1)
            for kc in range(KT):
                sync.dma_start(
                    out=xreg(3, kc), in_=xT[kc * P : (kc + 1) * P, 3 * TCH : 4 * TCH]
                ).then_inc(s_xc[2], 16)
            for e in range(NE_T0 + NE_T, NENT):
                ystore(e)
            for j in range(NYS):
                sync.wait_ge(s_ys[j], 16 * (NENT // NYS))

        @block.vector
        def _(vector):
            vector.wait_ge(s_tbl, 48)
            nc.vector.reciprocal(out=inv_s[:, :], in_=s_sb[:, :]).then_inc(s_dq, 1)
            vector.wait_ge(s_dq, 1)  # recip retired before stt reads inv_s
            for kc in range(KT):
                vector.wait_ge(s_wd[kc], 32)  # w8 + scales tiles landed
                sl = (kc % SCB) * O_PAD
                nc.vector.scalar_tensor_tensor(
                    wd(kc),
                    w8s[:, sl : sl + O_BF],
                    inv_s[:, kc : kc + 1],
                    scb[:, sl : sl + O_BF],
                    mybir.AluOpType.mult,
                    mybir.AluOpType.mult,
                ).then_inc(s_dq, 1)
                nc.vector.scalar_tensor_tensor(
                    w8d2[:, kc // 2, kc % 2, :],
                    w8s[:, sl + O_BF : sl + O_PAD],
                    inv_s[:, kc : kc + 1],
                    scb[:, sl + O_BF : sl + O_PAD],
                    mybir.AluOpType.mult,
                    mybir.AluOpType.mult,
                ).then_inc(s_dq, 1)
            # x -> fp8 conversions, chunk by chunk
            for kc in range(KT):
                vector.wait_ge(s_x0[kc], 16)
                nc.vector.tensor_scalar_mul(x8reg(0, kc), xreg(0, kc), 1.0).then_inc(
                    s_dq, 1
                )
            for tt in range(1, NT):
                vector.wait_ge(s_xc[tt - 1], 16 * KT)
                for kc in range(KT):
                    nc.vector.tensor_scalar_mul(
                        x8reg(tt, kc), xreg(tt, kc), 1.0
                    ).then_inc(s_dq, 1)

        @block.scalar
        def _(scalar):
            scalar.wait_ge(s_tbl, 48)
            for e in range(NENT):
                if e >= NYS:
                    scalar.wait_ge(s_ys[e % NYS], 16 * (e // NYS))
                scalar.wait_ge(s_pe, MM_END[e])
                if e < WAVE:
                    kind, tt, i = "bf", 0, e
                else:
                    kind, tt, i = SCHED[e - WAVE]
                if kind == "bf":
                    nc.scalar.activation(
                        yslot(e),
                        ps[e % NB][:, :],
                        mybir.ActivationFunctionType.Identity,
                        bias=bias_sb[:, i : i + 1],
                        scale=1.0,
                    ).then_inc(s_act, 1)
                else:
                    nc.scalar.activation(
                        yslot(e)[0:64, :],
                        ps[e % NB][0:64, :],
                        mybir.ActivationFunctionType.Identity,
                        bias=bias_dsb[:, i : i + 1],
                        scale=1.0,
                    ).then_inc(s_act, 1)

        @block.tensor
        def _(tensor):
            DR = mybir.MatmulPerfMode.DoubleRow
            # wave 1: groups (tt=0, ot=0..7) accumulate kc-major
            for kc in range(KT):
                tensor.wait_ge(s_x0[kc], 16)
                tensor.wait_ge(s_dq, dq_stt(kc))
                for b in range(WAVE):
                    nc.tensor.matmul(
                        ps[b][:, :],
                        wd(kc)[:, b * P : (b + 1) * P],
                        xreg(0, kc),
                        start=(kc == 0),
                        stop=(kc == KT - 1),
                    ).then_inc(s_pe, 1)
            # post-wave entries, sequential
            for ei, (kind, tt, i) in enumerate(SCHED):
                e = WAVE + ei
                if kind == "bf":
                    if i == 0 and tt > 0:
                        tensor.wait_ge(s_xc[tt - 1], 16 * KT)
                else:
                    if i == 0:
                        tensor.wait_ge(s_dq, dq_conv(tt))  # x8 chunk ready
                tensor.wait_ge(s_act, e - NB + 1)  # psum bank recycled
                if kind == "bf":
                    for kc in range(KT):
                        nc.tensor.matmul(
                            ps[e % NB][:, :],
                            wd(kc)[:, i * P : (i + 1) * P],
                            xreg(tt, kc),
                            start=(kc == 0),
                            stop=(kc == KT - 1),
                        ).then_inc(s_pe, 1)
                else:
                    for kp in range(KTP):
                        nc.tensor.matmul(
                            ps[e % NB][0:64, :],
                            w8d2[:, kp, :, i * 64 : (i + 1) * 64],
                            x8pair(tt, kp),
                            start=(kp == 0),
                            stop=(kp == KTP - 1),
                            perf_mode=DR,
                        ).then_inc(s_pe, 1)

    return nc


def get_nc():
    if "nc" not in _NC_CACHE:
        _NC_CACHE["nc"] = _build_nc()
    return _NC_CACHE["nc"]


def _prep_inputs(x, w_q, scales, s, bias):
    import ml_dtypes

    bf16 = ml_dtypes.bfloat16
    fp8 = ml_dtypes.float8_e4m3
    x = np.asarray(x, dtype=np.float32)
    w_q = np.asarray(w_q)
    scales = np.asarray(scales, dtype=np.float32)
    s = np.asarray(s, dtype=np.float32)
    bias = np.asarray(bias, dtype=np.float32)

    pad = O_PAD - O_SHARD  # 32 rows of zero-padding per shard
    # weights: int in [-7,7] -> fp8e4m3 exact
    w = w_q.reshape(OUT, IN).astype(fp8)
    sc = scales.reshape(OUT, N_GROUPS)  # f32

    xT = np.ascontiguousarray(x.T.astype(bf16))  # [IN, TOKENS] bf16
    s_cols = np.ascontiguousarray(s.reshape(KT, P).T)  # [128, 32] f32

    in_maps = []
    for c in range(N_CORES):
        lo, hi = c * O_SHARD, (c + 1) * O_SHARD
        w_c = np.pad(w[lo:hi], ((0, pad), (0, 0)))  # [O_PAD, IN] fp8
        sc_c = np.pad(sc[lo:hi], ((0, pad), (0, 0)))  # [O_PAD, 32]
        b_c = np.pad(bias[lo:hi], (0, pad))  # [O_PAD]
        in_maps.append(
            {
                "xT": xT,
                "w8T": np.ascontiguousarray(w_c.T),  # [IN, O_PAD] fp8
                "sc_repl": np.ascontiguousarray(
                    np.broadcast_to(
                        sc_c.T.astype(bf16)[:, None, :], (N_GROUPS, P, O_PAD)
                    )
                ),  # [32, 128, O_PAD] bf16 (layout move only)
                "s_cols": s_cols,
                "bias_cols": np.ascontiguousarray(
                    b_c.reshape(OT, P).T
                ),  # [128, 11] f32
                "bias_dr": np.ascontiguousarray(
                    b_c[O_BF:].reshape(DRH, 64).T
                ),  # [64, 4] f32
            }
        )
    return in_maps


def _install_profile_shim():
    """Provide antenv.axon_hooks (NTFF profiling via libaxon ctypes) when
    the container image lacks it. Only used for local perf iteration."""
    import contextlib
    import ctypes
    import sys
    import types

    if "antenv.axon_hooks" in sys.modules:
        return
    so_path = "/opt/axon/libaxon_pjrt.so"
    try:
        lib = ctypes.CDLL(so_path)
    except OSError:
        return
    if not hasattr(lib, "axon_start_nrt_profile"):
        return
    lib.axon_start_nrt_profile.argtypes = [
        ctypes.POINTER(ctypes.c_int64),
        ctypes.c_size_t,
    ]
    lib.axon_start_nrt_profile.restype = ctypes.c_int64
    lib.axon_stop_nrt_profile.argtypes = [ctypes.c_char_p]
    lib.axon_stop_nrt_profile.restype = ctypes.c_int64

    @contextlib.contextmanager
    def _hook(output_dir, device_ids):
        import jax

        jax.devices()
        if device_ids:
            ids = (ctypes.c_int64 * len(device_ids))(*device_ids)
            rc = lib.axon_start_nrt_profile(ids, len(device_ids))
        else:
            rc = lib.axon_start_nrt_profile(None, 0)
        if rc != 0:
            raise RuntimeError(f"axon_start_nrt_profile rc={rc}")
        try:
            yield
        finally:
            n = lib.axon_stop_nrt_profile(str(output_dir).encode())
            print(f"profile: {n} file(s) written to {output_dir}", file=sys.stderr)

    mod = types.ModuleType("antenv.axon_hooks")
    mod.get_axon_ntff_profile_hook = lambda: _hook
    mod.set_axon_ntff_profile_hook = lambda h: None
    sys.modules["antenv.axon_hooks"] = mod


def kernel(x, w_q, scales, s, bias):
    import sys

    if "/opt/trn_rl_repo" not in sys.path:
        sys.path.insert(0, "/opt/trn_rl_repo")
    import concourse.bass_utils as bass_utils
    from concourse.bass_utils import run_bass_kernel_spmd

    orig_dtype = np.asarray(x).dtype
    in_maps = _prep_inputs(x, w_q, scales, s, bias)
    nc = get_nc()

    trace = bool(os.environ.get("AWQ_TRACE"))
    kwargs = {}
    if trace:
        _install_profile_shim()
        bass_utils.upload_artifacts = lambda d: d  # zero-egress container
        tmpdir = os.environ.get("AWQ_TRACE_DIR")
        if tmpdir:
            os.makedirs(tmpdir, exist_ok=True)
            kwargs["tmpdir"] = tmpdir
    res = run_bass_kernel_spmd(
        nc,
        in_maps,
        core_ids=list(range(N_CORES)),
        trace=trace,
        **kwargs,
    )
    LAST["exec_time_ns"] = res.exec_time_ns
    LAST["results"] = res

    yT_full = np.concatenate(
        [np.asarray(res.results[c]["yT"], dtype=np.float32) for c in range(N_CORES)],
        axis=0,
    )  # [8*1408, 2048] f32
    y = np.ascontiguousarray(
        yT_full.reshape(N_CORES, O_PAD, TOKENS)[:, :O_SHARD, :]
        .reshape(OUT, TOKENS)
        .T
    )
    return y.astype(orig_dtype)

